# revision 30
# baseline (speedup 1.0000x reference)
"""Dilated-attention (segmented FlashMHA) for Trainium2, 8-core data parallel.

Problem (hardcoded): x [2, 8192, 1024], SEGMENT=2048, DILATION=2, 16 heads.
Each (batch, segment) pair is an independent attention problem over the
L = 1024 dilated tokens; there are exactly B * n_seg = 2 * 4 = 8 of them,
one per NeuronCore.  Weights are replicated.

v3 (bf16 + pair-interleaved attention).  Tolerance is 2e-2 and an
all-bf16 matmul pipeline measures ~7e-3 max-rel error, so every matmul
runs bf16 (fp32 PSUM accumulate).  The host pre-casts weights to bf16 and
pre-transposes xs (removes the on-device transpose phase, halves HBM
traffic); all weights are DMA'd up-front and stay SBUF-resident.

HW lessons baked in (A/B measured on device via loop-differencing):
- back-to-back bf16 MMs N=512 run at the 2.4 GHz model rate (211 ns);
  switching the stationary operand costs ~26 ns extra per MM.
- a lone sps->exp->ctx chain per chunk is catastrophically slow (~12 us
  per chunk): the PE sequencer is strict-FIFO, so a ctx MM waiting on
  ACT's exp blocks everything behind it.  Interleaving the two
  independent heads of a pair (they share qt) plus one projection-unit
  filler per chunk keeps >=1.7 us of issued-but-independent PE work
  between every exp and its consumer.

Per-core kernel:
  qkT  = Wqkv[:, :2048].T @ xsT  (+bias)   q/k transposed [dim, token]
  v    = xsT.T-contract @ Wv    (+bias)    token-major, head-blocked with
                                           a ones column per head (v_aug)
  per pair (heads A,B share qt):
    chunk c: sT_A/B = k_A/B . q (4 MMs), eT = exp(sT/8) via ACT ([128,512]
    tiles, bf16), filler projection sub-unit, ctx_A/B accumulate
    ([65,1024] PSUM each; row 64 = softmax denominator via ones column)
  normalize: ctxT = ctx[0:64] * recip(denom), per token-half (DVE+POOL)
  out  = ctxT.T-contract @ Wout + bout     fp32 out
"""

from contextlib import ExitStack

import numpy as np
import ml_dtypes

from concourse import bacc, bass_utils, mybir, tile
from concourse._compat import with_exitstack

F32 = mybir.dt.float32
BF16 = mybir.dt.bfloat16
AF = mybir.ActivationFunctionType
NPBF16 = ml_dtypes.bfloat16

B = 2
S = 8192
D = 1024
SEGMENT = 2048
DILATION = 2
N_SEG = S // SEGMENT          # 4
L = SEGMENT // DILATION       # 1024 tokens per (b, seg)
H = 16
HD = 64
NQK = 2048
SCALE = 0.125                 # 1 / sqrt(HD)
N_CORES = 8

_CACHE = {}


def _build(n_cores=N_CORES, loop_n=1):
    nc = bacc.Bacc("TRN2", debug=False, num_devices=n_cores)

    xsT_d = nc.dram_tensor("xsT", (D, L), BF16, kind="ExternalInput")
    wqk_d = nc.dram_tensor("wqk", (NQK, D), BF16, kind="ExternalInput")
    wv_d = nc.dram_tensor("wv", (2 * 128, 8 * 512), BF16, kind="ExternalInput")
    wout_d = nc.dram_tensor("wout", (D, D), BF16, kind="ExternalInput")
    bqkv_d = nc.dram_tensor("bqkv", (3 * D,), F32, kind="ExternalInput")
    bout_d = nc.dram_tensor("bout", (D,), F32, kind="ExternalInput")
    out_d = nc.dram_tensor("out", (L, D), F32, kind="ExternalOutput")

    with tile.TileContext(nc) as tc:
        if loop_n > 1:
            with tc.For_i(0, loop_n, 1):
                _emit(tc, out_d.ap(), xsT_d.ap(), wqk_d.ap(), wv_d.ap(),
                      wout_d.ap(), bqkv_d.ap(), bout_d.ap())
        else:
            _emit(tc, out_d.ap(), xsT_d.ap(), wqk_d.ap(), wv_d.ap(),
                  wout_d.ap(), bqkv_d.ap(), bout_d.ap())
    nc.compile()
    return nc


@with_exitstack
def _emit(ctx: ExitStack, tc, out, xsT_dram, wqk_dram, wv_dram, wout_dram,
          bqkv, bout):
    nc = tc.nc

    const_p = ctx.enter_context(tc.tile_pool(name="const", bufs=1))
    ctxT_p = ctx.enter_context(tc.tile_pool(name="ctxT", bufs=8))
    # PSUM pool for projection fillers (qk / v / out); 2 bufs so a unit's
    # matmuls overlap the previous unit's eviction
    proj_ps = ctx.enter_context(
        tc.tile_pool(name="proj_ps", bufs=2, space="PSUM"))

    # f32 constants
    bqk = const_p.tile([128, 16], F32)
    nc.sync.dma_start(out=bqk[:], in_=bqkv[0:NQK].rearrange("(c p) -> p c", p=128))
    # bias tiles broadcast across partitions (0-stride DRAM read); scalar
    # queue keeps them off the critical gpsimd weight queue
    bv_bc = const_p.tile([128, D], F32)
    bout_bc = const_p.tile([128, D], F32)
    ones16 = const_p.tile([128, 16], F32)
    nc.vector.memset(ones16[:], 1.0)
    # partition masks: col 0 selects rows 0:64, col 1 selects rows 64:128
    pmask = const_p.tile([128, 2], F32)
    nc.vector.memset(pmask[0:HD, 0:1], 1.0)
    nc.vector.memset(pmask[HD:128, 0:1], 0.0)
    nc.vector.memset(pmask[0:HD, 1:2], 0.0)
    nc.vector.memset(pmask[HD:128, 1:2], 1.0)

    ctxT = [ctxT_p.tile([128, L], BF16, tag="ctxT", name=f"ctxT{i}")
            for i in range(8)]

    with tc.tile_pool(name="xsT", bufs=8) as xsT_p, \
         tc.tile_pool(name="vaug", bufs=8) as vaug_p, \
         tc.tile_pool(name="qkT", bufs=6) as qkT_p, \
         tc.tile_pool(name="wqk", bufs=16) as wqk_p, \
         tc.tile_pool(name="wv", bufs=2) as wv_p, \
         tc.tile_pool(name="wout", bufs=8) as wo_p:

        # ---- up-front DMA of all inputs (weights fully resident); xsT is
        # on the critical path (first qk unit contracts all 8 r-tiles), so
        # split it across two queues
        xsT = [xsT_p.tile([128, L], BF16, tag="xsT", name=f"xsT{r}")
               for r in range(8)]
        for r in range(0, 8, 2):
            nc.sync.dma_start(out=xsT[r][:],
                              in_=xsT_dram[r * 128:(r + 1) * 128, :])
            nc.scalar.dma_start(out=xsT[r + 1][:],
                                in_=xsT_dram[(r + 1) * 128:(r + 2) * 128, :])
        wqk = [wqk_p.tile([128, D], BF16, tag="w", name=f"wqk{m}")
               for m in range(16)]
        wv = [wv_p.tile([128, 8 * 512], BF16, tag="wv", name=f"wv{q}")
              for q in range(2)]
        wo = []

        def dma_wqk(m):
            nc.gpsimd.dma_start(out=wqk[m][:],
                                in_=wqk_dram[m * 128:(m + 1) * 128, :])

        def dma_wv(q):
            nc.gpsimd.dma_start(out=wv[q][:],
                                in_=wv_dram[q * 128:(q + 1) * 128, :])

        # order: pair-0 weights first, then v halves, then the rest
        dma_wqk(0)
        dma_wqk(8)
        dma_wv(0)
        nc.gpsimd.dma_start(out=bv_bc[:],
                            in_=bqkv[NQK:3 * D].partition_broadcast(128))
        dma_wqk(1)
        dma_wqk(9)
        dma_wv(1)
        for p in range(2, 8):
            dma_wqk(p)
            dma_wqk(8 + p)
        nc.gpsimd.dma_start(out=bout_bc[:], in_=bout.partition_broadcast(128))
        for r in range(8):
            t = wo_p.tile([128, D], BF16, tag="wo", name=f"wo{r}")
            nc.scalar.dma_start(
                out=t[:], in_=wout_dram[r * 128:(r + 1) * 128, :])
            wo.append(t)

        vaug = [vaug_p.tile([128, H * (HD + 1)], BF16, tag="vaug",
                            name=f"vaug{i}") for i in range(8)]
        for l in range(8):
            dst = vaug[l][:].rearrange("p (h e) -> p h e", e=HD + 1)
            nc.vector.tensor_copy(out=dst[:, :, HD:HD + 1],
                                  in_=ones16[:].unsqueeze(2))

        # ---------- emission helpers --------------------------------------
        # Each projection unit is 8 accumulation MMs + a DVE eviction.
        # parts() returns (A, B) closures splitting the unit into two 4-MM
        # halves sharing one PSUM buffer, so filler slots can be 0.85 us
        # grains; unit() is the fused form.
        def _mk(parts_fn):
            def unit():
                a, b = parts_fn()
                a()
                b()
            def parts():
                return parts_fn()
            unit.parts = parts
            return unit

        def emit_qk_half(m, dest, half):
            """qkT row-tile m (dims m*128..), token half -> dest[:, half]."""
            def parts_fn():
                st = {}
                def mms(r0, r1):
                    for r in range(r0, r1):
                        nc.tensor.matmul(
                            st["ps"][:], wqk[m][:, r * 128:(r + 1) * 128],
                            xsT[r][:, half * 512:(half + 1) * 512],
                            start=(r == 0), stop=(r == 7),
                        )
                def a():
                    st["ps"] = proj_ps.tile([128, 512], F32, tag="proj",
                                            name="ps")
                    mms(0, 4)
                def b():
                    mms(4, 8)
                    nc.vector.tensor_scalar_add(
                        out=dest[:, half * 512:(half + 1) * 512],
                        in0=st["ps"][:], scalar1=bqk[:, m:m + 1])
                return a, b
            return _mk(parts_fn)

        def emit_k_half(m, dest0, dest1, half):
            """k row-tile m, token half -> two zero-padded per-head tiles.

            dest0 keeps rows 0:64 (head 2p) and zeroes rows 64:128;
            dest1 keeps rows 64:128 (head 2p+1) and zeroes rows 0:64, so
            score matmuls contract K=128 from base partition 0.
            """
            def parts_fn():
                st = {}
                def mms(r0, r1):
                    for r in range(r0, r1):
                        nc.tensor.matmul(
                            st["ps"][:], wqk[m][:, r * 128:(r + 1) * 128],
                            xsT[r][:, half * 512:(half + 1) * 512],
                            start=(r == 0), stop=(r == 7),
                        )
                def a():
                    st["ps"] = proj_ps.tile([128, 512], F32, tag="proj",
                                            name="ps")
                    mms(0, 4)
                def b():
                    mms(4, 8)
                    for dest, mc in ((dest0, 0), (dest1, 1)):
                        nc.vector.tensor_scalar(
                            out=dest[:, half * 512:(half + 1) * 512],
                            in0=st["ps"][:], scalar1=bqk[:, m:m + 1],
                            scalar2=pmask[:, mc:mc + 1],
                            op0=mybir.AluOpType.add,
                            op1=mybir.AluOpType.mult)
                return a, b
            return _mk(parts_fn)

        def emit_v_unit(q, l):
            """v half q (heads 8q..8q+7), token chunk l -> vaug[l]."""
            def parts_fn():
                st = {}
                def mms(r0, r1):
                    for r in range(r0, r1):
                        nc.tensor.matmul(
                            st["ps"][:], xsT[r][:, l * 128:(l + 1) * 128],
                            wv[q][:, r * 512:(r + 1) * 512],
                            start=(r == 0), stop=(r == 7),
                        )
                def a():
                    st["ps"] = proj_ps.tile([128, 512], F32, tag="proj",
                                            name="vps")
                    mms(0, 4)
                def b():
                    mms(4, 8)
                    dst = vaug[l][:].rearrange("p (h e) -> p h e", e=HD + 1)
                    nc.vector.tensor_tensor(
                        out=dst[:, q * 8:(q + 1) * 8, 0:HD],
                        in0=st["ps"][:].rearrange("p (h e) -> p h e", e=HD),
                        in1=bv_bc[:].rearrange("p (h e) -> p h e", e=HD)[
                            :, q * 8:(q + 1) * 8, :],
                        op=mybir.AluOpType.add,
                    )
                return a, b
            return _mk(parts_fn)

        def sub2(*units):
            """Flatten units into their A/B sub-closures."""
            out_ = []
            for u in units:
                a, b = u.parts()
                out_ += [a, b]
            return out_

        # ---- prelude: q pair0 (both halves), k pair0 half 0, vaug[0] ----
        qk_tiles = {}
        qk_tiles[0] = (qkT_p.tile([128, L], BF16, tag="qkT", name="qt0"),
                       qkT_p.tile([128, L], BF16, tag="qkT", name="kt0a"),
                       qkT_p.tile([128, L], BF16, tag="qkT", name="kt0b"))
        emit_qk_half(0, qk_tiles[0][0], 0)()
        emit_qk_half(0, qk_tiles[0][0], 1)()
        emit_k_half(8, qk_tiles[0][1], qk_tiles[0][2], 0)()
        emit_v_unit(0, 0)()

        def new_qk_tiles(p):
            t = (qkT_p.tile([128, L], BF16, tag="qkT", name=f"qt{p}"),
                 qkT_p.tile([128, L], BF16, tag="qkT", name=f"kt{p}a"),
                 qkT_p.tile([128, L], BF16, tag="qkT", name=f"kt{p}b"))
            qk_tiles[p] = t
            return t

        # ---- attention heads with interleaved fillers (v2 structure) -----
        with tc.tile_pool(name="expT", bufs=4) as exp_p, \
             tc.tile_pool(name="craw", bufs=2) as craw_p, \
             tc.tile_pool(name="srow", bufs=2) as srow_p, \
             tc.tile_pool(name="rbc", bufs=2) as rbc_p, \
             tc.tile_pool(name="o_sb", bufs=6) as o_sb, \
             tc.tile_pool(name="opart", bufs=14) as opart_p, \
             tc.tile_pool(name="s_ps", bufs=2, space="PSUM") as s_ps, \
             tc.tile_pool(name="c_ps", bufs=1, space="PSUM") as c_ps:

            # out-projection units; the first two get prefilled (r=0..6)
            # as pair-7 fillers, completing after ctxT[7] lands
            out_state = {}
            # r=0..3 partial sums of the remaining out units run as fillers
            # in the bare late heads, staged through SBUF
            oparts = {}

            def emit_out_part(l, half, r0, r1):
                def part():
                    if r0 == 0:
                        out_state[(l, half)] = proj_ps.tile(
                            [128, 512], F32, tag="proj", name="ops")
                    ps = out_state[(l, half)]
                    for r in range(r0, r1):
                        nc.tensor.matmul(
                            ps[:], ctxT[r][:, l * 128:(l + 1) * 128],
                            wo[r][:, half * 512:(half + 1) * 512],
                            start=(r == 0), stop=(r == 7),
                        )
                    if r1 == 8:
                        osb = o_sb.tile([128, 512], F32, tag="osb",
                                        name="osb")
                        nc.vector.tensor_tensor(
                            out=osb[:], in0=ps[:],
                            in1=bout_bc[:, half * 512:(half + 1) * 512],
                            op=mybir.AluOpType.add)
                        nc.sync.dma_start(
                            out=out[l * 128:(l + 1) * 128,
                                    half * 512:(half + 1) * 512],
                            in_=osb[:],
                        )
                return part

            def emit_out_stage1(l, half):
                """r=0..3 half-contraction of out unit (l, half) -> SBUF."""
                def unit():
                    ps = proj_ps.tile([128, 512], F32, tag="proj",
                                      name="o1ps")
                    for r in range(4):
                        nc.tensor.matmul(
                            ps[:], ctxT[r][:, l * 128:(l + 1) * 128],
                            wo[r][:, half * 512:(half + 1) * 512],
                            start=(r == 0), stop=(r == 3),
                        )
                    op = opart_p.tile([128, 512], BF16, tag="opart",
                                      name="opart")
                    nc.vector.tensor_copy(out=op[:], in_=ps[:])
                    oparts[(l, half)] = op
                return unit

            def emit_out_stage2(l, half):
                ps = proj_ps.tile([128, 512], F32, tag="proj", name="o2ps")
                for r in range(4, 8):
                    nc.tensor.matmul(
                        ps[:], ctxT[r][:, l * 128:(l + 1) * 128],
                        wo[r][:, half * 512:(half + 1) * 512],
                        start=(r == 4), stop=(r == 7),
                    )
                t1 = o_sb.tile([128, 512], F32, tag="osb", name="t1")
                nc.vector.tensor_tensor(
                    out=t1[:], in0=ps[:], in1=oparts[(l, half)][:],
                    op=mybir.AluOpType.add)
                osb = o_sb.tile([128, 512], F32, tag="osb", name="osb")
                nc.vector.tensor_tensor(
                    out=osb[:], in0=t1[:],
                    in1=bout_bc[:, half * 512:(half + 1) * 512],
                    op=mybir.AluOpType.add)
                nc.sync.dma_start(
                    out=out[l * 128:(l + 1) * 128,
                            half * 512:(half + 1) * 512],
                    in_=osb[:],
                )

            fillers_iters = {}
            for h in range(H):
                p = h // 2
                po = (h % 2) * HD
                qt = qk_tiles[p][0]
                ktp = qk_tiles[p][1 + (h % 2)]

                if h == 0:
                    fillers_iters[0] = iter(
                        [emit_v_unit(0, 1), emit_v_unit(0, 2),
                         emit_v_unit(0, 3),
                         emit_k_half(8, qk_tiles[0][1], qk_tiles[0][2], 1),
                         emit_v_unit(0, 4), emit_v_unit(0, 5),
                         emit_v_unit(0, 6), emit_v_unit(0, 7)])
                elif h == 1:
                    n1 = new_qk_tiles(1)
                    fillers_iters[0] = iter(
                        [emit_qk_half(1, n1[0], 0),
                         emit_qk_half(1, n1[0], 1),
                         emit_k_half(9, n1[1], n1[2], 0),
                         emit_k_half(9, n1[1], n1[2], 1)] +
                        [emit_v_unit(1, l) for l in range(4)])
                elif h == 3:
                    # v half-1 units deferred from head 1: covers this
                    # otherwise-bare head (deadline: before head 8)
                    fillers_iters[1] = iter(
                        [emit_v_unit(1, l) for l in range(4, 8)])
                elif h % 2 == 0 and p < 7:
                    nx = new_qk_tiles(p + 1)
                    fillers_iters[p] = iter(
                        [emit_qk_half(p + 1, nx[0], 0),
                         emit_qk_half(p + 1, nx[0], 1),
                         emit_k_half(9 + p, nx[1], nx[2], 0),
                         emit_k_half(9 + p, nx[1], nx[2], 1)])
                elif h == 14:
                    fillers_iters[7] = iter(
                        [emit_out_part(0, 0, 0, 4), emit_out_part(0, 0, 4, 7),
                         emit_out_part(0, 1, 0, 4), emit_out_part(0, 1, 4, 7)])
                elif h in (9, 11, 13):
                    # bare odd heads of pairs 4-6: out r=0..3 partials
                    k0 = {9: 0, 11: 5, 13: 10}[h]
                    units = [(1 + i // 2, i % 2) for i in range(14)]
                    fillers_iters[p] = iter(
                        [emit_out_stage1(l, hf) for l, hf in
                         units[k0:k0 + (5 if h < 13 else 4)]])
                fillers_iter = fillers_iters[p]

                cps = c_ps.tile([128, L], F32, tag="cps", name="cps")
                for c in range(8):
                    sps = s_ps.tile([128, L], F32, tag="sps", name="sps")
                    for half in range(2):
                        nc.tensor.matmul(
                            sps[:, half * 512:(half + 1) * 512],
                            ktp[:, c * 128:(c + 1) * 128],
                            qt[:, half * 512:(half + 1) * 512],
                            start=True, stop=True,
                        )
                    et = exp_p.tile([128, L], BF16, tag="expT", name="et")
                    nc.scalar.activation(out=et[:], in_=sps[:], func=AF.Exp,
                                         scale=SCALE)
                    u = next(fillers_iter, None)
                    if u is not None:
                        u()
                    for half in range(2):
                        nc.tensor.matmul(
                            cps[0:HD + 1, half * 512:(half + 1) * 512],
                            vaug[c][:, h * (HD + 1):(h + 1) * (HD + 1)],
                            et[:, half * 512:(half + 1) * 512],
                            start=(c == 0), stop=(c == 7),
                        )
                # normalize: PSUM-freeing copy + recip first, leftover
                # fillers next, POOL broadcast + multiply last.  The final
                # head normalizes per token-half so the prefilled out units
                # (which only read ctxT[7][:, 0:128]) unblock sooner.
                craw = craw_p.tile([HD + 1, L], F32, tag="craw", name="craw")
                if h == H - 1:
                    rec = srow_p.tile([1, L], F32, tag="srow", name="rec")
                    rbc = rbc_p.tile([HD, L], F32, tag="rbc", name="rbc")
                    for m in range(2):
                        sl = slice(m * 512, (m + 1) * 512)
                        nc.vector.tensor_copy(out=craw[:, sl],
                                              in_=cps[0:HD + 1, sl])
                        nc.vector.reciprocal(out=rec[:, sl],
                                             in_=craw[HD:HD + 1, sl])
                        nc.gpsimd.partition_broadcast(rbc[:, sl],
                                                      rec[:, sl])
                        nc.vector.tensor_mul(
                            ctxT[p][po:po + HD, sl], craw[0:HD, sl],
                            rbc[:, sl])
                    del qk_tiles[p]
                    continue
                nc.vector.tensor_copy(out=craw[:], in_=cps[0:HD + 1, :])
                rec = srow_p.tile([1, L], F32, tag="srow", name="rec")
                nc.vector.reciprocal(out=rec[:], in_=craw[HD:HD + 1, :])
                if h % 2 == 1:
                    for u in fillers_iter:   # drain leftovers
                        u()
                rbc = rbc_p.tile([HD, L], F32, tag="rbc", name="rbc")
                nc.gpsimd.partition_broadcast(rbc[:], rec[:])
                nc.vector.tensor_mul(
                    ctxT[p][po:po + HD, :], craw[0:HD, :], rbc[:])
                if h % 2 == 1:
                    del qk_tiles[p]

            # ---- out = ctxT.T-contract @ Wout + bout ---------------------
            for l in range(8):
                for half in range(2):
                    if (l, half) in out_state:
                        emit_out_part(l, half, 7, 8)()
                    elif (l, half) in oparts:
                        emit_out_stage2(l, half)
                    else:
                        emit_out_part(l, half, 0, 8)()


def get_nc():
    if "nc" not in _CACHE:
        _CACHE["nc"] = _build()
    return _CACHE["nc"]


def make_in_maps(x, Wqkv, bqkv, Wout, bout):
    """Shard: core i -> (batch i//N_SEG, segment i%N_SEG), dilated tokens.

    Host-side prep: cast to bf16, pre-transpose xs, and lay weights out
    contraction-major so every DMA is a contiguous [128, N] row-tile.
    """
    x = np.asarray(x, dtype=np.float32)
    Wqkv = np.asarray(Wqkv, dtype=np.float32)
    bqkv = np.ascontiguousarray(np.asarray(bqkv, dtype=np.float32))
    Wout = np.asarray(Wout, dtype=np.float32)
    bout = np.ascontiguousarray(np.asarray(bout, dtype=np.float32))

    wqkv_bf = Wqkv.astype(NPBF16)
    # [16 m, 128 p, 8 r, 128 c] -> [2048, 1024]: row-tile m is contiguous
    wqk = np.ascontiguousarray(
        wqkv_bf[:, :NQK].reshape(8, 128, 16, 128).transpose(2, 1, 0, 3)
    ).reshape(NQK, D)
    # [2 q, 128 p, 8 r, 512 c] -> [256, 4096]
    wv = np.ascontiguousarray(
        wqkv_bf[:, NQK:].reshape(8, 128, 2, 512).transpose(2, 1, 0, 3)
    ).reshape(256, 4096)
    wout = np.ascontiguousarray(Wout.astype(NPBF16))

    in_maps = []
    for i in range(N_CORES):
        b, seg = divmod(i, N_SEG)
        xs = x[b, seg * SEGMENT:(seg + 1) * SEGMENT:DILATION, :]
        xsT = np.ascontiguousarray(xs.T.astype(NPBF16))
        in_maps.append({"xsT": xsT, "wqk": wqk, "wv": wv, "wout": wout,
                        "bqkv": bqkv, "bout": bout})
    return in_maps


def unshard(results):
    out = np.empty((B, N_SEG * L, D), dtype=np.float32)
    for i in range(N_CORES):
        b, seg = divmod(i, N_SEG)
        out[b, seg * L:(seg + 1) * L, :] = results[i]["out"]
    return out


def kernel(x, Wqkv, bqkv, Wout, bout):
    nc = get_nc()
    in_maps = make_in_maps(x, Wqkv, bqkv, Wout, bout)
    res = bass_utils.run_bass_kernel_spmd(nc, in_maps,
                                          core_ids=list(range(N_CORES)))
    return unshard(res.results)


# revision 31
# speedup vs baseline: 1.1868x; 1.1868x over previous
"""Dilated-attention (segmented FlashMHA) for Trainium2, 8-core data parallel.

Problem (hardcoded): x [2, 8192, 1024], SEGMENT=2048, DILATION=2, 16 heads.
Each (batch, segment) pair is an independent attention problem over the
L = 1024 dilated tokens; there are exactly B * n_seg = 2 * 4 = 8 of them,
one per NeuronCore.  Weights are replicated.

v3 (bf16 + pair-interleaved attention).  Tolerance is 2e-2 and an
all-bf16 matmul pipeline measures ~7e-3 max-rel error, so every matmul
runs bf16 (fp32 PSUM accumulate).  The host pre-casts weights to bf16 and
pre-transposes xs (removes the on-device transpose phase, halves HBM
traffic); all weights are DMA'd up-front and stay SBUF-resident.

HW lessons baked in (A/B measured on device via loop-differencing):
- back-to-back bf16 MMs N=512 run at the 2.4 GHz model rate (211 ns);
  switching the stationary operand costs ~26 ns extra per MM.
- a lone sps->exp->ctx chain per chunk is catastrophically slow (~12 us
  per chunk): the PE sequencer is strict-FIFO, so a ctx MM waiting on
  ACT's exp blocks everything behind it.  Interleaving the two
  independent heads of a pair (they share qt) plus one projection-unit
  filler per chunk keeps >=1.7 us of issued-but-independent PE work
  between every exp and its consumer.

Per-core kernel:
  qkT  = Wqkv[:, :2048].T @ xsT  (+bias)   q/k transposed [dim, token]
  v    = xsT.T-contract @ Wv    (+bias)    token-major, head-blocked with
                                           a ones column per head (v_aug)
  per pair (heads A,B share qt):
    chunk c: sT_A/B = k_A/B . q (4 MMs), eT = exp(sT/8) via ACT ([128,512]
    tiles, bf16), filler projection sub-unit, ctx_A/B accumulate
    ([65,1024] PSUM each; row 64 = softmax denominator via ones column)
  normalize: ctxT = ctx[0:64] * recip(denom), per token-half (DVE+POOL)
  out  = ctxT.T-contract @ Wout + bout     fp32 out
"""

from contextlib import ExitStack

import numpy as np
import ml_dtypes

from concourse import bacc, bass_utils, mybir, tile
from concourse._compat import with_exitstack

F32 = mybir.dt.float32
BF16 = mybir.dt.bfloat16
AF = mybir.ActivationFunctionType
NPBF16 = ml_dtypes.bfloat16

B = 2
S = 8192
D = 1024
SEGMENT = 2048
DILATION = 2
N_SEG = S // SEGMENT          # 4
L = SEGMENT // DILATION       # 1024 tokens per (b, seg)
H = 16
HD = 64
NQK = 2048
SCALE = 0.125                 # 1 / sqrt(HD)
N_CORES = 8

_CACHE = {}


def _build(n_cores=N_CORES, loop_n=1):
    nc = bacc.Bacc("TRN2", debug=False, num_devices=n_cores)

    xsT_d = nc.dram_tensor("xsT", (D, L), BF16, kind="ExternalInput")
    wqk_d = nc.dram_tensor("wqk", (NQK, D), BF16, kind="ExternalInput")
    wv_d = nc.dram_tensor("wv", (2 * 128, 8 * 512), BF16, kind="ExternalInput")
    wout_d = nc.dram_tensor("wout", (D, D), BF16, kind="ExternalInput")
    bqkv_d = nc.dram_tensor("bqkv", (3 * D,), F32, kind="ExternalInput")
    bout_d = nc.dram_tensor("bout", (D,), F32, kind="ExternalInput")
    out_d = nc.dram_tensor("out", (L, D), F32, kind="ExternalOutput")

    with tile.TileContext(nc) as tc:
        if loop_n > 1:
            with tc.For_i(0, loop_n, 1):
                _emit(tc, out_d.ap(), xsT_d.ap(), wqk_d.ap(), wv_d.ap(),
                      wout_d.ap(), bqkv_d.ap(), bout_d.ap())
        else:
            _emit(tc, out_d.ap(), xsT_d.ap(), wqk_d.ap(), wv_d.ap(),
                  wout_d.ap(), bqkv_d.ap(), bout_d.ap())
    nc.compile()
    return nc


@with_exitstack
def _emit(ctx: ExitStack, tc, out, xsT_dram, wqk_dram, wv_dram, wout_dram,
          bqkv, bout):
    nc = tc.nc

    const_p = ctx.enter_context(tc.tile_pool(name="const", bufs=1))
    ctxT_p = ctx.enter_context(tc.tile_pool(name="ctxT", bufs=8))
    # PSUM pool for projection fillers (qk / v / out); 2 bufs so a unit's
    # matmuls overlap the previous unit's eviction
    proj_ps = ctx.enter_context(
        tc.tile_pool(name="proj_ps", bufs=2, space="PSUM"))

    # f32 constants
    bqk = const_p.tile([128, 16], F32)
    nc.sync.dma_start(out=bqk[:], in_=bqkv[0:NQK].rearrange("(c p) -> p c", p=128))
    # bias tiles broadcast across partitions (0-stride DRAM read); scalar
    # queue keeps them off the critical gpsimd weight queue
    bv_bc = const_p.tile([128, D], F32)
    bout_bc = const_p.tile([128, D], F32)
    ones16 = const_p.tile([128, 16], F32)
    nc.vector.memset(ones16[:], 1.0)
    # partition masks: col 0 selects rows 0:64, col 1 selects rows 64:128
    pmask = const_p.tile([128, 2], F32)
    nc.vector.memset(pmask[0:HD, 0:1], 1.0)
    nc.vector.memset(pmask[HD:128, 0:1], 0.0)
    nc.vector.memset(pmask[0:HD, 1:2], 0.0)
    nc.vector.memset(pmask[HD:128, 1:2], 1.0)

    ctxT = [ctxT_p.tile([128, L], BF16, tag="ctxT", name=f"ctxT{i}")
            for i in range(8)]

    with tc.tile_pool(name="xsT", bufs=8) as xsT_p, \
         tc.tile_pool(name="vaug", bufs=8) as vaug_p, \
         tc.tile_pool(name="qkT", bufs=6) as qkT_p, \
         tc.tile_pool(name="wqk", bufs=16) as wqk_p, \
         tc.tile_pool(name="wv", bufs=2) as wv_p, \
         tc.tile_pool(name="wout", bufs=8) as wo_p:

        # ---- up-front DMA of all inputs (weights fully resident); xsT is
        # on the critical path (first qk unit contracts all 8 r-tiles), so
        # split it across two queues
        xsT = [xsT_p.tile([128, L], BF16, tag="xsT", name=f"xsT{r}")
               for r in range(8)]
        for r in range(0, 8, 2):
            nc.sync.dma_start(out=xsT[r][:],
                              in_=xsT_dram[r * 128:(r + 1) * 128, :])
            nc.scalar.dma_start(out=xsT[r + 1][:],
                                in_=xsT_dram[(r + 1) * 128:(r + 2) * 128, :])
        wqk = [wqk_p.tile([128, D], BF16, tag="w", name=f"wqk{m}")
               for m in range(16)]
        wv = [wv_p.tile([128, 8 * 512], BF16, tag="wv", name=f"wv{q}")
              for q in range(2)]
        wo = []

        def dma_wqk(m):
            nc.gpsimd.dma_start(out=wqk[m][:],
                                in_=wqk_dram[m * 128:(m + 1) * 128, :])

        def dma_wv(q):
            nc.gpsimd.dma_start(out=wv[q][:],
                                in_=wv_dram[q * 128:(q + 1) * 128, :])

        # order: pair-0 weights first, then v halves, then the rest
        dma_wqk(0)
        dma_wqk(8)
        dma_wv(0)
        nc.gpsimd.dma_start(out=bv_bc[:],
                            in_=bqkv[NQK:3 * D].partition_broadcast(128))
        dma_wqk(1)
        dma_wqk(9)
        dma_wv(1)
        for p in range(2, 8):
            dma_wqk(p)
            dma_wqk(8 + p)
        nc.gpsimd.dma_start(out=bout_bc[:], in_=bout.partition_broadcast(128))
        for r in range(8):
            t = wo_p.tile([128, D], BF16, tag="wo", name=f"wo{r}")
            nc.scalar.dma_start(
                out=t[:], in_=wout_dram[r * 128:(r + 1) * 128, :])
            wo.append(t)

        vaug = [vaug_p.tile([128, H * (HD + 1)], BF16, tag="vaug",
                            name=f"vaug{i}") for i in range(8)]
        for l in range(8):
            dst = vaug[l][:].rearrange("p (h e) -> p h e", e=HD + 1)
            nc.vector.tensor_copy(out=dst[:, :, HD:HD + 1],
                                  in_=ones16[:].unsqueeze(2))

        # ---------- emission helpers --------------------------------------
        # Each projection unit is 8 accumulation MMs + a DVE eviction.
        # parts() returns (A, B) closures splitting the unit into two 4-MM
        # halves sharing one PSUM buffer, so filler slots can be 0.85 us
        # grains; unit() is the fused form.
        def _mk(parts_fn):
            def unit():
                a, b = parts_fn()
                a()
                b()
            def parts():
                return parts_fn()
            unit.parts = parts
            return unit

        def emit_qk_half(m, dest, half):
            """qkT row-tile m (dims m*128..), token half -> dest[:, half]."""
            def parts_fn():
                st = {}
                def mms(r0, r1):
                    for r in range(r0, r1):
                        nc.tensor.matmul(
                            st["ps"][:], wqk[m][:, r * 128:(r + 1) * 128],
                            xsT[r][:, half * 512:(half + 1) * 512],
                            start=(r == 0), stop=(r == 7),
                        )
                def a():
                    st["ps"] = proj_ps.tile([128, 512], F32, tag="proj",
                                            name="ps")
                    mms(0, 4)
                def b():
                    mms(4, 8)
                    nc.vector.tensor_scalar_add(
                        out=dest[:, half * 512:(half + 1) * 512],
                        in0=st["ps"][:], scalar1=bqk[:, m:m + 1])
                return a, b
            return _mk(parts_fn)

        def emit_k_half(m, dest0, dest1, half):
            """k row-tile m, token half -> two zero-padded per-head tiles.

            dest0 keeps rows 0:64 (head 2p) and zeroes rows 64:128;
            dest1 keeps rows 64:128 (head 2p+1) and zeroes rows 0:64, so
            score matmuls contract K=128 from base partition 0.
            """
            def parts_fn():
                st = {}
                def mms(r0, r1):
                    for r in range(r0, r1):
                        nc.tensor.matmul(
                            st["ps"][:], wqk[m][:, r * 128:(r + 1) * 128],
                            xsT[r][:, half * 512:(half + 1) * 512],
                            start=(r == 0), stop=(r == 7),
                        )
                def a():
                    st["ps"] = proj_ps.tile([128, 512], F32, tag="proj",
                                            name="ps")
                    mms(0, 4)
                def b():
                    mms(4, 8)
                    for dest, mc in ((dest0, 0), (dest1, 1)):
                        nc.vector.tensor_scalar(
                            out=dest[:, half * 512:(half + 1) * 512],
                            in0=st["ps"][:], scalar1=bqk[:, m:m + 1],
                            scalar2=pmask[:, mc:mc + 1],
                            op0=mybir.AluOpType.add,
                            op1=mybir.AluOpType.mult)
                return a, b
            return _mk(parts_fn)

        def emit_v_unit(q, l):
            """v half q (heads 8q..8q+7), token chunk l -> vaug[l]."""
            def parts_fn():
                st = {}
                def mms(r0, r1):
                    for r in range(r0, r1):
                        nc.tensor.matmul(
                            st["ps"][:], xsT[r][:, l * 128:(l + 1) * 128],
                            wv[q][:, r * 512:(r + 1) * 512],
                            start=(r == 0), stop=(r == 7),
                        )
                def a():
                    st["ps"] = proj_ps.tile([128, 512], F32, tag="proj",
                                            name="vps")
                    mms(0, 4)
                def b():
                    mms(4, 8)
                    dst = vaug[l][:].rearrange("p (h e) -> p h e", e=HD + 1)
                    nc.vector.tensor_tensor(
                        out=dst[:, q * 8:(q + 1) * 8, 0:HD],
                        in0=st["ps"][:].rearrange("p (h e) -> p h e", e=HD),
                        in1=bv_bc[:].rearrange("p (h e) -> p h e", e=HD)[
                            :, q * 8:(q + 1) * 8, :],
                        op=mybir.AluOpType.add,
                    )
                return a, b
            return _mk(parts_fn)

        def sub2(*units):
            """Flatten units into their A/B sub-closures."""
            out_ = []
            for u in units:
                a, b = u.parts()
                out_ += [a, b]
            return out_

        # ---- prelude: q pair0 (both halves), k pair0 half 0, vaug[0] ----
        qk_tiles = {}
        qk_tiles[0] = (qkT_p.tile([128, L], BF16, tag="qkT", name="qt0"),
                       qkT_p.tile([128, L], BF16, tag="qkT", name="kt0a"),
                       qkT_p.tile([128, L], BF16, tag="qkT", name="kt0b"))
        emit_qk_half(0, qk_tiles[0][0], 0)()
        emit_qk_half(0, qk_tiles[0][0], 1)()
        emit_k_half(8, qk_tiles[0][1], qk_tiles[0][2], 0)()
        emit_v_unit(0, 0)()

        def new_qk_tiles(p):
            t = (qkT_p.tile([128, L], BF16, tag="qkT", name=f"qt{p}"),
                 qkT_p.tile([128, L], BF16, tag="qkT", name=f"kt{p}a"),
                 qkT_p.tile([128, L], BF16, tag="qkT", name=f"kt{p}b"))
            qk_tiles[p] = t
            return t

        # ---- attention heads with interleaved fillers (v2 structure) -----
        with tc.tile_pool(name="expT", bufs=4) as exp_p, \
             tc.tile_pool(name="craw", bufs=2) as craw_p, \
             tc.tile_pool(name="srow", bufs=2) as srow_p, \
             tc.tile_pool(name="rbc", bufs=2) as rbc_p, \
             tc.tile_pool(name="o_sb", bufs=6) as o_sb, \
             tc.tile_pool(name="opart", bufs=14) as opart_p, \
             tc.tile_pool(name="s_ps", bufs=2, space="PSUM") as s_ps, \
             tc.tile_pool(name="c_ps", bufs=1, space="PSUM") as c_ps:

            # out-projection units; the first two get prefilled (r=0..6)
            # as pair-7 fillers, completing after ctxT[7] lands
            out_state = {}
            # r=0..3 partial sums of the remaining out units run as fillers
            # in the bare late heads, staged through SBUF
            oparts = {}

            def emit_out_part(l, half, r0, r1):
                def part():
                    if r0 == 0:
                        out_state[(l, half)] = proj_ps.tile(
                            [128, 512], F32, tag="proj", name="ops")
                    ps = out_state[(l, half)]
                    for r in range(r0, r1):
                        nc.tensor.matmul(
                            ps[:], ctxT[r][:, l * 128:(l + 1) * 128],
                            wo[r][:, half * 512:(half + 1) * 512],
                            start=(r == 0), stop=(r == 7),
                        )
                    if r1 == 8:
                        osb = o_sb.tile([128, 512], F32, tag="osb",
                                        name="osb")
                        nc.vector.tensor_tensor(
                            out=osb[:], in0=ps[:],
                            in1=bout_bc[:, half * 512:(half + 1) * 512],
                            op=mybir.AluOpType.add)
                        nc.sync.dma_start(
                            out=out[l * 128:(l + 1) * 128,
                                    half * 512:(half + 1) * 512],
                            in_=osb[:],
                        )
                return part

            def emit_out_stage1(l, half):
                """r=0..3 half-contraction of out unit (l, half) -> SBUF."""
                def unit():
                    ps = proj_ps.tile([128, 512], F32, tag="proj",
                                      name="o1ps")
                    for r in range(4):
                        nc.tensor.matmul(
                            ps[:], ctxT[r][:, l * 128:(l + 1) * 128],
                            wo[r][:, half * 512:(half + 1) * 512],
                            start=(r == 0), stop=(r == 3),
                        )
                    op = opart_p.tile([128, 512], BF16, tag="opart",
                                      name="opart")
                    nc.vector.tensor_tensor(
                        out=op[:], in0=ps[:],
                        in1=bout_bc[:, half * 512:(half + 1) * 512],
                        op=mybir.AluOpType.add)
                    oparts[(l, half)] = op
                return unit

            def emit_out_stage2(l, half):
                ps = proj_ps.tile([128, 512], F32, tag="proj", name="o2ps")
                for r in range(4, 8):
                    nc.tensor.matmul(
                        ps[:], ctxT[r][:, l * 128:(l + 1) * 128],
                        wo[r][:, half * 512:(half + 1) * 512],
                        start=(r == 4), stop=(r == 7),
                    )
                osb = o_sb.tile([128, 512], F32, tag="osb", name="osb")
                nc.vector.tensor_tensor(
                    out=osb[:], in0=ps[:], in1=oparts[(l, half)][:],
                    op=mybir.AluOpType.add)
                nc.sync.dma_start(
                    out=out[l * 128:(l + 1) * 128,
                            half * 512:(half + 1) * 512],
                    in_=osb[:],
                )

            fillers_iters = {}
            for h in range(H):
                p = h // 2
                po = (h % 2) * HD
                qt = qk_tiles[p][0]
                ktp = qk_tiles[p][1 + (h % 2)]

                if h == 0:
                    fillers_iters[0] = iter(
                        [emit_v_unit(0, 1), emit_v_unit(0, 2),
                         emit_v_unit(0, 3),
                         emit_k_half(8, qk_tiles[0][1], qk_tiles[0][2], 1),
                         emit_v_unit(0, 4), emit_v_unit(0, 5),
                         emit_v_unit(0, 6), emit_v_unit(0, 7)])
                elif h == 1:
                    n1 = new_qk_tiles(1)
                    fillers_iters[0] = iter(
                        [emit_qk_half(1, n1[0], 0),
                         emit_qk_half(1, n1[0], 1),
                         emit_k_half(9, n1[1], n1[2], 0),
                         emit_k_half(9, n1[1], n1[2], 1)] +
                        [emit_v_unit(1, l) for l in range(4)])
                elif h == 3:
                    # v half-1 units deferred from head 1: covers this
                    # otherwise-bare head (deadline: before head 8)
                    fillers_iters[1] = iter(
                        [emit_v_unit(1, l) for l in range(4, 8)])
                elif h % 2 == 0 and p < 7:
                    nx = new_qk_tiles(p + 1)
                    fillers_iters[p] = iter(
                        [emit_qk_half(p + 1, nx[0], 0),
                         emit_qk_half(p + 1, nx[0], 1),
                         emit_k_half(9 + p, nx[1], nx[2], 0),
                         emit_k_half(9 + p, nx[1], nx[2], 1)])
                elif h == 14:
                    fillers_iters[7] = iter(
                        [emit_out_part(0, 0, 0, 4), emit_out_part(0, 0, 4, 7),
                         emit_out_part(0, 1, 0, 4), emit_out_part(0, 1, 4, 7)])
                elif h in (9, 11, 13, 15):
                    # bare odd heads of pairs 4-7: out r=0..3 partials
                    k0, n = {9: (0, 4), 11: (4, 4), 13: (8, 3),
                             15: (11, 3)}[h]
                    units = [(1 + i // 2, i % 2) for i in range(14)]
                    fillers_iters[p] = iter(
                        [emit_out_stage1(l, hf) for l, hf in
                         units[k0:k0 + n]])
                fillers_iter = fillers_iters[p]

                cps = c_ps.tile([128, L], F32, tag="cps", name="cps")
                for c in range(8):
                    sps = s_ps.tile([128, L], F32, tag="sps", name="sps")
                    for half in range(2):
                        nc.tensor.matmul(
                            sps[:, half * 512:(half + 1) * 512],
                            ktp[:, c * 128:(c + 1) * 128],
                            qt[:, half * 512:(half + 1) * 512],
                            start=True, stop=True,
                        )
                    et = exp_p.tile([128, L], BF16, tag="expT", name="et")
                    nc.scalar.activation(out=et[:], in_=sps[:], func=AF.Exp,
                                         scale=SCALE)
                    u = next(fillers_iter, None)
                    if u is not None:
                        u()
                    for half in range(2):
                        nc.tensor.matmul(
                            cps[0:HD + 1, half * 512:(half + 1) * 512],
                            vaug[c][:, h * (HD + 1):(h + 1) * (HD + 1)],
                            et[:, half * 512:(half + 1) * 512],
                            start=(c == 0), stop=(c == 7),
                        )
                # normalize: PSUM-freeing copy + recip first, leftover
                # fillers next, POOL broadcast + multiply last.  The final
                # head normalizes per token-half so the prefilled out units
                # (which only read ctxT[7][:, 0:128]) unblock sooner.
                craw = craw_p.tile([HD + 1, L], F32, tag="craw", name="craw")
                if h == H - 1:
                    rec = srow_p.tile([1, L], F32, tag="srow", name="rec")
                    rbc = rbc_p.tile([HD, L], F32, tag="rbc", name="rbc")
                    for m in range(2):
                        sl = slice(m * 512, (m + 1) * 512)
                        nc.vector.tensor_copy(out=craw[:, sl],
                                              in_=cps[0:HD + 1, sl])
                        nc.vector.reciprocal(out=rec[:, sl],
                                             in_=craw[HD:HD + 1, sl])
                        nc.gpsimd.partition_broadcast(rbc[:, sl],
                                                      rec[:, sl])
                        nc.vector.tensor_mul(
                            ctxT[p][po:po + HD, sl], craw[0:HD, sl],
                            rbc[:, sl])
                    del qk_tiles[p]
                    continue
                nc.vector.tensor_copy(out=craw[:], in_=cps[0:HD + 1, :])
                rec = srow_p.tile([1, L], F32, tag="srow", name="rec")
                nc.vector.reciprocal(out=rec[:], in_=craw[HD:HD + 1, :])
                if h % 2 == 1:
                    for u in fillers_iter:   # drain leftovers
                        u()
                rbc = rbc_p.tile([HD, L], F32, tag="rbc", name="rbc")
                nc.gpsimd.partition_broadcast(rbc[:], rec[:])
                nc.vector.tensor_mul(
                    ctxT[p][po:po + HD, :], craw[0:HD, :], rbc[:])
                if h % 2 == 1:
                    del qk_tiles[p]

            # ---- out = ctxT.T-contract @ Wout + bout ---------------------
            for l in range(8):
                for half in range(2):
                    if (l, half) in out_state:
                        emit_out_part(l, half, 7, 8)()
                    elif (l, half) in oparts:
                        emit_out_stage2(l, half)
                    else:
                        emit_out_part(l, half, 0, 8)()


def get_nc():
    if "nc" not in _CACHE:
        _CACHE["nc"] = _build()
    return _CACHE["nc"]


def make_in_maps(x, Wqkv, bqkv, Wout, bout):
    """Shard: core i -> (batch i//N_SEG, segment i%N_SEG), dilated tokens.

    Host-side prep: cast to bf16, pre-transpose xs, and lay weights out
    contraction-major so every DMA is a contiguous [128, N] row-tile.
    """
    x = np.asarray(x, dtype=np.float32)
    Wqkv = np.asarray(Wqkv, dtype=np.float32)
    bqkv = np.ascontiguousarray(np.asarray(bqkv, dtype=np.float32))
    Wout = np.asarray(Wout, dtype=np.float32)
    bout = np.ascontiguousarray(np.asarray(bout, dtype=np.float32))

    wqkv_bf = Wqkv.astype(NPBF16)
    # [16 m, 128 p, 8 r, 128 c] -> [2048, 1024]: row-tile m is contiguous
    wqk = np.ascontiguousarray(
        wqkv_bf[:, :NQK].reshape(8, 128, 16, 128).transpose(2, 1, 0, 3)
    ).reshape(NQK, D)
    # [2 q, 128 p, 8 r, 512 c] -> [256, 4096]
    wv = np.ascontiguousarray(
        wqkv_bf[:, NQK:].reshape(8, 128, 2, 512).transpose(2, 1, 0, 3)
    ).reshape(256, 4096)
    wout = np.ascontiguousarray(Wout.astype(NPBF16))

    in_maps = []
    for i in range(N_CORES):
        b, seg = divmod(i, N_SEG)
        xs = x[b, seg * SEGMENT:(seg + 1) * SEGMENT:DILATION, :]
        xsT = np.ascontiguousarray(xs.T.astype(NPBF16))
        in_maps.append({"xsT": xsT, "wqk": wqk, "wv": wv, "wout": wout,
                        "bqkv": bqkv, "bout": bout})
    return in_maps


def unshard(results):
    out = np.empty((B, N_SEG * L, D), dtype=np.float32)
    for i in range(N_CORES):
        b, seg = divmod(i, N_SEG)
        out[b, seg * L:(seg + 1) * L, :] = results[i]["out"]
    return out


def kernel(x, Wqkv, bqkv, Wout, bout):
    nc = get_nc()
    in_maps = make_in_maps(x, Wqkv, bqkv, Wout, bout)
    res = bass_utils.run_bass_kernel_spmd(nc, in_maps,
                                          core_ids=list(range(N_CORES)))
    return unshard(res.results)


# revision 32
# speedup vs baseline: 1.2045x; 1.0149x over previous
"""Dilated-attention (segmented FlashMHA) for Trainium2, 8-core data parallel.

Problem (hardcoded): x [2, 8192, 1024], SEGMENT=2048, DILATION=2, 16 heads.
Each (batch, segment) pair is an independent attention problem over the
L = 1024 dilated tokens; there are exactly B * n_seg = 2 * 4 = 8 of them,
one per NeuronCore.  Weights are replicated.

v11 (bf16, filler-scheduled).  Tolerance is 2e-2 and an all-bf16 matmul
pipeline measures ~7e-3 max-rel error, so every matmul runs bf16 (fp32
PSUM accumulate).  The host pre-casts weights to bf16 and pre-transposes
xs (removes the on-device transpose phase, halves HBM traffic); all
weights are DMA'd up-front and stay SBUF-resident.

HW lessons baked in (A/B measured on device via loop-differencing):
- back-to-back bf16 MMs N=512 run at the 2.4 GHz model rate (211 ns);
  switching the stationary operand costs ~26 ns extra per MM.
- a lone sps->exp->ctx chain per chunk is catastrophically slow (~12 us
  per chunk): the PE sequencer is strict-FIFO, so a ctx MM waiting on
  ACT's exp blocks everything behind it.  A projection-unit filler
  between each chunk's exp and ctx keeps issued-but-independent PE work
  in front of every dependent matmul; out-projection r=0..3 partials
  (staged through SBUF) supply fillers for the late heads that have no
  projection work left.

Per-core kernel:
  qkT  = Wqkv[:, :2048].T @ xsT  (+bias)   q/k transposed [dim, token]
  v    = xsT.T-contract @ Wv    (+bias)    token-major, head-blocked with
                                           a ones column per head (v_aug)
  per head h, chunk c: sT = k_h . q (2 MMs), eT = exp(sT/8) via ACT
    ([128,1024] tiles, bf16 out), one filler unit, ctx accumulate
    ([65,1024] PSUM; row 64 = softmax denominator via the ones column)
  normalize: ctxT = ctx[0:64] * recip(denom) (DVE+POOL broadcast); the
    final head normalizes per token-half to unblock the prefilled tail
  out  = ctxT.T-contract @ Wout + bout     fp32 out (l=0 prefilled r0..6
    during head 14; l>=1 as stage1 partial + stage2 completion)
"""

from contextlib import ExitStack

import numpy as np
import ml_dtypes

from concourse import bacc, bass_utils, mybir, tile
from concourse._compat import with_exitstack

F32 = mybir.dt.float32
BF16 = mybir.dt.bfloat16
AF = mybir.ActivationFunctionType
NPBF16 = ml_dtypes.bfloat16

B = 2
S = 8192
D = 1024
SEGMENT = 2048
DILATION = 2
N_SEG = S // SEGMENT          # 4
L = SEGMENT // DILATION       # 1024 tokens per (b, seg)
H = 16
HD = 64
NQK = 2048
SCALE = 0.125                 # 1 / sqrt(HD)
N_CORES = 8

_CACHE = {}


def _build(n_cores=N_CORES, loop_n=1):
    nc = bacc.Bacc("TRN2", debug=False, num_devices=n_cores)

    xsT_d = nc.dram_tensor("xsT", (D, L), BF16, kind="ExternalInput")
    wqk_d = nc.dram_tensor("wqk", (NQK, D), BF16, kind="ExternalInput")
    wv_d = nc.dram_tensor("wv", (2 * 128, 8 * 512), BF16, kind="ExternalInput")
    wout_d = nc.dram_tensor("wout", (D, D), BF16, kind="ExternalInput")
    bqkv_d = nc.dram_tensor("bqkv", (3 * D,), F32, kind="ExternalInput")
    bout_d = nc.dram_tensor("bout", (D,), F32, kind="ExternalInput")
    out_d = nc.dram_tensor("out", (L, D), F32, kind="ExternalOutput")

    with tile.TileContext(nc) as tc:
        if loop_n > 1:
            with tc.For_i(0, loop_n, 1):
                _emit(tc, out_d.ap(), xsT_d.ap(), wqk_d.ap(), wv_d.ap(),
                      wout_d.ap(), bqkv_d.ap(), bout_d.ap())
        else:
            _emit(tc, out_d.ap(), xsT_d.ap(), wqk_d.ap(), wv_d.ap(),
                  wout_d.ap(), bqkv_d.ap(), bout_d.ap())
    nc.compile()
    return nc


@with_exitstack
def _emit(ctx: ExitStack, tc, out, xsT_dram, wqk_dram, wv_dram, wout_dram,
          bqkv, bout):
    nc = tc.nc

    const_p = ctx.enter_context(tc.tile_pool(name="const", bufs=1))
    ctxT_p = ctx.enter_context(tc.tile_pool(name="ctxT", bufs=8))
    # PSUM pool for projection fillers (qk / v / out); 2 bufs so a unit's
    # matmuls overlap the previous unit's eviction
    proj_ps = ctx.enter_context(
        tc.tile_pool(name="proj_ps", bufs=2, space="PSUM"))

    # f32 constants
    bqk = const_p.tile([128, 16], F32)
    nc.sync.dma_start(out=bqk[:], in_=bqkv[0:NQK].rearrange("(c p) -> p c", p=128))
    # bias tiles broadcast across partitions (0-stride DRAM read); scalar
    # queue keeps them off the critical gpsimd weight queue
    bv_bc = const_p.tile([128, D], F32)
    bout_bc = const_p.tile([128, D], F32)
    ones16 = const_p.tile([128, 16], F32)
    nc.vector.memset(ones16[:], 1.0)
    # partition masks: col 0 selects rows 0:64, col 1 selects rows 64:128
    pmask = const_p.tile([128, 2], F32)
    nc.vector.memset(pmask[0:HD, 0:1], 1.0)
    nc.vector.memset(pmask[HD:128, 0:1], 0.0)
    nc.vector.memset(pmask[0:HD, 1:2], 0.0)
    nc.vector.memset(pmask[HD:128, 1:2], 1.0)

    ctxT = [ctxT_p.tile([128, L], BF16, tag="ctxT", name=f"ctxT{i}")
            for i in range(8)]

    with tc.tile_pool(name="xsT", bufs=8) as xsT_p, \
         tc.tile_pool(name="vaug", bufs=8) as vaug_p, \
         tc.tile_pool(name="qkT", bufs=6) as qkT_p, \
         tc.tile_pool(name="wqk", bufs=16) as wqk_p, \
         tc.tile_pool(name="wv", bufs=2) as wv_p, \
         tc.tile_pool(name="wout", bufs=8) as wo_p:

        # ---- up-front DMA of all inputs (weights fully resident); xsT is
        # on the critical path (first qk unit contracts all 8 r-tiles), so
        # split it across two queues
        xsT = [xsT_p.tile([128, L], BF16, tag="xsT", name=f"xsT{r}")
               for r in range(8)]
        for r in range(0, 8, 2):
            nc.sync.dma_start(out=xsT[r][:],
                              in_=xsT_dram[r * 128:(r + 1) * 128, :])
            nc.scalar.dma_start(out=xsT[r + 1][:],
                                in_=xsT_dram[(r + 1) * 128:(r + 2) * 128, :])
        wqk = [wqk_p.tile([128, D], BF16, tag="w", name=f"wqk{m}")
               for m in range(16)]
        wv = [wv_p.tile([128, 8 * 512], BF16, tag="wv", name=f"wv{q}")
              for q in range(2)]
        wo = []

        def dma_wqk(m):
            nc.gpsimd.dma_start(out=wqk[m][:],
                                in_=wqk_dram[m * 128:(m + 1) * 128, :])

        def dma_wv(q):
            nc.gpsimd.dma_start(out=wv[q][:],
                                in_=wv_dram[q * 128:(q + 1) * 128, :])

        # order: pair-0 weights first, then v halves, then the rest
        dma_wqk(0)
        dma_wqk(8)
        dma_wv(0)
        nc.gpsimd.dma_start(out=bv_bc[:],
                            in_=bqkv[NQK:3 * D].partition_broadcast(128))
        dma_wqk(1)
        dma_wqk(9)
        dma_wv(1)
        for p in range(2, 8):
            dma_wqk(p)
            dma_wqk(8 + p)
        nc.gpsimd.dma_start(out=bout_bc[:], in_=bout.partition_broadcast(128))
        for r in range(8):
            t = wo_p.tile([128, D], BF16, tag="wo", name=f"wo{r}")
            nc.scalar.dma_start(
                out=t[:], in_=wout_dram[r * 128:(r + 1) * 128, :])
            wo.append(t)

        vaug = [vaug_p.tile([128, H * (HD + 1)], BF16, tag="vaug",
                            name=f"vaug{i}") for i in range(8)]
        for l in range(8):
            dst = vaug[l][:].rearrange("p (h e) -> p h e", e=HD + 1)
            nc.vector.tensor_copy(out=dst[:, :, HD:HD + 1],
                                  in_=ones16[:].unsqueeze(2))

        # ---------- emission helpers --------------------------------------
        # Each projection unit is 8 accumulation MMs + a DVE eviction.
        # parts() returns (A, B) closures splitting the unit into two 4-MM
        # halves sharing one PSUM buffer, so filler slots can be 0.85 us
        # grains; unit() is the fused form.
        def _mk(parts_fn):
            def unit():
                a, b = parts_fn()
                a()
                b()
            def parts():
                return parts_fn()
            unit.parts = parts
            return unit

        def emit_qk_half(m, dest, half):
            """qkT row-tile m (dims m*128..), token half -> dest[:, half]."""
            def parts_fn():
                st = {}
                def mms(r0, r1):
                    for r in range(r0, r1):
                        nc.tensor.matmul(
                            st["ps"][:], wqk[m][:, r * 128:(r + 1) * 128],
                            xsT[r][:, half * 512:(half + 1) * 512],
                            start=(r == 0), stop=(r == 7),
                        )
                def a():
                    st["ps"] = proj_ps.tile([128, 512], F32, tag="proj",
                                            name="ps")
                    mms(0, 4)
                def b():
                    mms(4, 8)
                    nc.vector.tensor_scalar_add(
                        out=dest[:, half * 512:(half + 1) * 512],
                        in0=st["ps"][:], scalar1=bqk[:, m:m + 1])
                return a, b
            return _mk(parts_fn)

        def emit_k_half(m, dest0, dest1, half):
            """k row-tile m, token half -> two zero-padded per-head tiles.

            dest0 keeps rows 0:64 (head 2p) and zeroes rows 64:128;
            dest1 keeps rows 64:128 (head 2p+1) and zeroes rows 0:64, so
            score matmuls contract K=128 from base partition 0.
            """
            def parts_fn():
                st = {}
                def mms(r0, r1):
                    for r in range(r0, r1):
                        nc.tensor.matmul(
                            st["ps"][:], wqk[m][:, r * 128:(r + 1) * 128],
                            xsT[r][:, half * 512:(half + 1) * 512],
                            start=(r == 0), stop=(r == 7),
                        )
                def a():
                    st["ps"] = proj_ps.tile([128, 512], F32, tag="proj",
                                            name="ps")
                    mms(0, 4)
                def b():
                    mms(4, 8)
                    for dest, mc in ((dest0, 0), (dest1, 1)):
                        nc.vector.tensor_scalar(
                            out=dest[:, half * 512:(half + 1) * 512],
                            in0=st["ps"][:], scalar1=bqk[:, m:m + 1],
                            scalar2=pmask[:, mc:mc + 1],
                            op0=mybir.AluOpType.add,
                            op1=mybir.AluOpType.mult)
                return a, b
            return _mk(parts_fn)

        def emit_v_unit(q, l):
            """v half q (heads 8q..8q+7), token chunk l -> vaug[l]."""
            def parts_fn():
                st = {}
                def mms(r0, r1):
                    for r in range(r0, r1):
                        nc.tensor.matmul(
                            st["ps"][:], xsT[r][:, l * 128:(l + 1) * 128],
                            wv[q][:, r * 512:(r + 1) * 512],
                            start=(r == 0), stop=(r == 7),
                        )
                def a():
                    st["ps"] = proj_ps.tile([128, 512], F32, tag="proj",
                                            name="vps")
                    mms(0, 4)
                def b():
                    mms(4, 8)
                    dst = vaug[l][:].rearrange("p (h e) -> p h e", e=HD + 1)
                    nc.vector.tensor_tensor(
                        out=dst[:, q * 8:(q + 1) * 8, 0:HD],
                        in0=st["ps"][:].rearrange("p (h e) -> p h e", e=HD),
                        in1=bv_bc[:].rearrange("p (h e) -> p h e", e=HD)[
                            :, q * 8:(q + 1) * 8, :],
                        op=mybir.AluOpType.add,
                    )
                return a, b
            return _mk(parts_fn)

        def sub2(*units):
            """Flatten units into their A/B sub-closures."""
            out_ = []
            for u in units:
                a, b = u.parts()
                out_ += [a, b]
            return out_

        # ---- prelude: q pair0 (both halves), k pair0 half 0, vaug[0] ----
        qk_tiles = {}
        qk_tiles[0] = (qkT_p.tile([128, L], BF16, tag="qkT", name="qt0"),
                       qkT_p.tile([128, L], BF16, tag="qkT", name="kt0a"),
                       qkT_p.tile([128, L], BF16, tag="qkT", name="kt0b"))
        emit_qk_half(0, qk_tiles[0][0], 0)()
        emit_qk_half(0, qk_tiles[0][0], 1)()
        emit_k_half(8, qk_tiles[0][1], qk_tiles[0][2], 0)()
        emit_v_unit(0, 0)()

        def new_qk_tiles(p):
            t = (qkT_p.tile([128, L], BF16, tag="qkT", name=f"qt{p}"),
                 qkT_p.tile([128, L], BF16, tag="qkT", name=f"kt{p}a"),
                 qkT_p.tile([128, L], BF16, tag="qkT", name=f"kt{p}b"))
            qk_tiles[p] = t
            return t

        # ---- attention heads with interleaved fillers (v2 structure) -----
        with tc.tile_pool(name="expT", bufs=4) as exp_p, \
             tc.tile_pool(name="craw", bufs=2) as craw_p, \
             tc.tile_pool(name="srow", bufs=2) as srow_p, \
             tc.tile_pool(name="rbc", bufs=2) as rbc_p, \
             tc.tile_pool(name="o_sb", bufs=6) as o_sb, \
             tc.tile_pool(name="opart", bufs=14) as opart_p, \
             tc.tile_pool(name="s_ps", bufs=2, space="PSUM") as s_ps, \
             tc.tile_pool(name="c_ps", bufs=1, space="PSUM") as c_ps:

            # out-projection units; the first two get prefilled (r=0..6)
            # as pair-7 fillers, completing after ctxT[7] lands
            out_state = {}
            # r=0..3 partial sums of the remaining out units run as fillers
            # in the bare late heads, staged through SBUF
            oparts = {}

            def emit_out_part(l, half, r0, r1):
                def part():
                    if r0 == 0:
                        out_state[(l, half)] = proj_ps.tile(
                            [128, 512], F32, tag="proj", name="ops")
                    ps = out_state[(l, half)]
                    for r in range(r0, r1):
                        nc.tensor.matmul(
                            ps[:], ctxT[r][:, l * 128:(l + 1) * 128],
                            wo[r][:, half * 512:(half + 1) * 512],
                            start=(r == 0), stop=(r == 7),
                        )
                    if r1 == 8:
                        osb = o_sb.tile([128, 512], F32, tag="osb",
                                        name="osb")
                        nc.vector.tensor_tensor(
                            out=osb[:], in0=ps[:],
                            in1=bout_bc[:, half * 512:(half + 1) * 512],
                            op=mybir.AluOpType.add)
                        nc.sync.dma_start(
                            out=out[l * 128:(l + 1) * 128,
                                    half * 512:(half + 1) * 512],
                            in_=osb[:],
                        )
                return part

            def emit_out_stage1(l, half):
                """r=0..3 half-contraction of out unit (l, half) -> SBUF."""
                def unit():
                    ps = proj_ps.tile([128, 512], F32, tag="proj",
                                      name="o1ps")
                    for r in range(4):
                        nc.tensor.matmul(
                            ps[:], ctxT[r][:, l * 128:(l + 1) * 128],
                            wo[r][:, half * 512:(half + 1) * 512],
                            start=(r == 0), stop=(r == 3),
                        )
                    op = opart_p.tile([128, 512], BF16, tag="opart",
                                      name="opart")
                    nc.vector.tensor_tensor(
                        out=op[:], in0=ps[:],
                        in1=bout_bc[:, half * 512:(half + 1) * 512],
                        op=mybir.AluOpType.add)
                    oparts[(l, half)] = op
                return unit

            def emit_out_stage2(l, half):
                ps = proj_ps.tile([128, 512], F32, tag="proj", name="o2ps")
                for r in range(4, 8):
                    nc.tensor.matmul(
                        ps[:], ctxT[r][:, l * 128:(l + 1) * 128],
                        wo[r][:, half * 512:(half + 1) * 512],
                        start=(r == 4), stop=(r == 7),
                    )
                osb = o_sb.tile([128, 512], F32, tag="osb", name="osb")
                nc.vector.tensor_tensor(
                    out=osb[:], in0=ps[:], in1=oparts[(l, half)][:],
                    op=mybir.AluOpType.add)
                nc.sync.dma_start(
                    out=out[l * 128:(l + 1) * 128,
                            half * 512:(half + 1) * 512],
                    in_=osb[:],
                )

            fillers_iters = {}
            for h in range(H):
                p = h // 2
                po = (h % 2) * HD
                qt = qk_tiles[p][0]
                ktp = qk_tiles[p][1 + (h % 2)]

                if h == 0:
                    fillers_iters[0] = iter(
                        [emit_v_unit(0, 1), emit_v_unit(0, 2),
                         emit_v_unit(0, 3),
                         emit_k_half(8, qk_tiles[0][1], qk_tiles[0][2], 1),
                         emit_v_unit(0, 4), emit_v_unit(0, 5),
                         emit_v_unit(0, 6), emit_v_unit(0, 7)])
                elif h == 1:
                    n1 = new_qk_tiles(1)
                    fillers_iters[0] = iter(
                        [emit_qk_half(1, n1[0], 0),
                         emit_qk_half(1, n1[0], 1),
                         emit_k_half(9, n1[1], n1[2], 0),
                         emit_k_half(9, n1[1], n1[2], 1)] +
                        [emit_v_unit(1, l) for l in range(4)])
                elif h == 3:
                    # v half-1 units deferred from head 1: covers this
                    # otherwise-bare head (deadline: before head 8)
                    fillers_iters[1] = iter(
                        [emit_v_unit(1, l) for l in range(4, 8)])
                elif h % 2 == 0 and p < 7:
                    nx = new_qk_tiles(p + 1)
                    fillers_iters[p] = iter(
                        [emit_qk_half(p + 1, nx[0], 0),
                         emit_qk_half(p + 1, nx[0], 1),
                         emit_k_half(9 + p, nx[1], nx[2], 0),
                         emit_k_half(9 + p, nx[1], nx[2], 1)])
                elif h == 14:
                    fillers_iters[7] = iter(
                        [emit_out_part(0, 0, 0, 4), emit_out_part(0, 0, 4, 7),
                         emit_out_part(0, 1, 0, 4), emit_out_part(0, 1, 4, 7)])
                elif h in (9, 11, 13, 15):
                    # bare odd heads of pairs 4-7: out r=0..3 partials
                    k0, n = {9: (0, 4), 11: (4, 4), 13: (8, 3),
                             15: (11, 3)}[h]
                    units = [(1 + i // 2, i % 2) for i in range(14)]
                    fillers_iters[p] = iter(
                        [emit_out_stage1(l, hf) for l, hf in
                         units[k0:k0 + n]])
                fillers_iter = fillers_iters[p]

                cps = c_ps.tile([128, L], F32, tag="cps", name="cps")
                for c in range(8):
                    sps = s_ps.tile([128, L], F32, tag="sps", name="sps")
                    for half in range(2):
                        nc.tensor.matmul(
                            sps[:, half * 512:(half + 1) * 512],
                            ktp[:, c * 128:(c + 1) * 128],
                            qt[:, half * 512:(half + 1) * 512],
                            start=True, stop=True,
                        )
                    et = exp_p.tile([128, L], BF16, tag="expT", name="et")
                    nc.scalar.activation(out=et[:], in_=sps[:], func=AF.Exp,
                                         scale=SCALE)
                    u = next(fillers_iter, None)
                    if u is not None:
                        u()
                    for half in range(2):
                        nc.tensor.matmul(
                            cps[0:HD + 1, half * 512:(half + 1) * 512],
                            vaug[c][:, h * (HD + 1):(h + 1) * (HD + 1)],
                            et[:, half * 512:(half + 1) * 512],
                            start=(c == 0), stop=(c == 7),
                        )
                # normalize: PSUM-freeing copy + recip first, leftover
                # fillers next, POOL broadcast + multiply last.  The final
                # head normalizes per token-half so the prefilled out units
                # (which only read ctxT[7][:, 0:128]) unblock sooner.
                craw = craw_p.tile([HD + 1, L], F32, tag="craw", name="craw")
                if h == H - 1:
                    rec = srow_p.tile([1, L], F32, tag="srow", name="rec")
                    rbc = rbc_p.tile([HD, L], F32, tag="rbc", name="rbc")
                    for m in range(2):
                        sl = slice(m * 512, (m + 1) * 512)
                        nc.vector.tensor_copy(out=craw[:, sl],
                                              in_=cps[0:HD + 1, sl])
                        nc.vector.reciprocal(out=rec[:, sl],
                                             in_=craw[HD:HD + 1, sl])
                        nc.gpsimd.partition_broadcast(rbc[:, sl],
                                                      rec[:, sl])
                        nc.vector.tensor_mul(
                            ctxT[p][po:po + HD, sl], craw[0:HD, sl],
                            rbc[:, sl])
                    del qk_tiles[p]
                    continue
                nc.vector.tensor_copy(out=craw[:], in_=cps[0:HD + 1, :])
                rec = srow_p.tile([1, L], F32, tag="srow", name="rec")
                nc.vector.reciprocal(out=rec[:], in_=craw[HD:HD + 1, :])
                if h % 2 == 1:
                    for u in fillers_iter:   # drain leftovers
                        u()
                rbc = rbc_p.tile([HD, L], F32, tag="rbc", name="rbc")
                nc.gpsimd.partition_broadcast(rbc[:], rec[:])
                nc.vector.tensor_mul(
                    ctxT[p][po:po + HD, :], craw[0:HD, :], rbc[:])
                if h % 2 == 1:
                    del qk_tiles[p]

            # ---- out = ctxT.T-contract @ Wout + bout ---------------------
            for l in range(8):
                for half in range(2):
                    if (l, half) in out_state:
                        emit_out_part(l, half, 7, 8)()
                    elif (l, half) in oparts:
                        emit_out_stage2(l, half)
                    else:
                        emit_out_part(l, half, 0, 8)()


def get_nc():
    if "nc" not in _CACHE:
        _CACHE["nc"] = _build()
    return _CACHE["nc"]


def make_in_maps(x, Wqkv, bqkv, Wout, bout):
    """Shard: core i -> (batch i//N_SEG, segment i%N_SEG), dilated tokens.

    Host-side prep: cast to bf16, pre-transpose xs, and lay weights out
    contraction-major so every DMA is a contiguous [128, N] row-tile.
    """
    x = np.asarray(x, dtype=np.float32)
    Wqkv = np.asarray(Wqkv, dtype=np.float32)
    bqkv = np.ascontiguousarray(np.asarray(bqkv, dtype=np.float32))
    Wout = np.asarray(Wout, dtype=np.float32)
    bout = np.ascontiguousarray(np.asarray(bout, dtype=np.float32))

    wqkv_bf = Wqkv.astype(NPBF16)
    # [16 m, 128 p, 8 r, 128 c] -> [2048, 1024]: row-tile m is contiguous
    wqk = np.ascontiguousarray(
        wqkv_bf[:, :NQK].reshape(8, 128, 16, 128).transpose(2, 1, 0, 3)
    ).reshape(NQK, D)
    # [2 q, 128 p, 8 r, 512 c] -> [256, 4096]
    wv = np.ascontiguousarray(
        wqkv_bf[:, NQK:].reshape(8, 128, 2, 512).transpose(2, 1, 0, 3)
    ).reshape(256, 4096)
    wout = np.ascontiguousarray(Wout.astype(NPBF16))

    in_maps = []
    for i in range(N_CORES):
        b, seg = divmod(i, N_SEG)
        xs = x[b, seg * SEGMENT:(seg + 1) * SEGMENT:DILATION, :]
        xsT = np.ascontiguousarray(xs.T.astype(NPBF16))
        in_maps.append({"xsT": xsT, "wqk": wqk, "wv": wv, "wout": wout,
                        "bqkv": bqkv, "bout": bout})
    return in_maps


def unshard(results):
    out = np.empty((B, N_SEG * L, D), dtype=np.float32)
    for i in range(N_CORES):
        b, seg = divmod(i, N_SEG)
        out[b, seg * L:(seg + 1) * L, :] = results[i]["out"]
    return out


def kernel(x, Wqkv, bqkv, Wout, bout):
    nc = get_nc()
    in_maps = make_in_maps(x, Wqkv, bqkv, Wout, bout)
    res = bass_utils.run_bass_kernel_spmd(nc, in_maps,
                                          core_ids=list(range(N_CORES)))
    return unshard(res.results)
